# revision 4
# baseline (speedup 1.0000x reference)
"""Multi-head latent attention on 8 Trainium2 NeuronCores (Bass/Tile).

Takes the FULL inputs (B=2048, S=32, D=1024, L=256, H=16), shards the batch
across 8 cores (data parallel, weights replicated), runs a fused Bass kernel
per core, and gathers the full outputs.

Host-side algebraic folding: (k+v) @ W_down == x @ ((W_k+W_v) @ W_down), so
the two big K/V projections collapse into one [D, L] GEMM. The 0.5 scale is
folded into the fused weight and into a pre-scaled copy of `memory`.

Per-core kernel layout strategy: activations are produced feature-major
([d_part, token]) for matmul inputs; v and the final out are produced
token-major by swapping which operand is stationary. Transposes (x, latent,
attn, ctx) go through the PE with an identity. Attention works per
(batch, head) pair with PE sub-array placement (tile_position) and dense PSUM
packing; softmax normalizes pre-transpose (no max-subtraction needed: |scores|
is bounded ~1.5 for these input scales). Projection matmuls run in f32r
(full-rate fp32 mode); attention + out-projection in bf16 with fp32 PSUM.
"""
from contextlib import ExitStack

import ml_dtypes
import numpy as np

import concourse.bass as bass
import concourse.mybir as mybir
from concourse import tile
from concourse.bass_utils import run_bass_kernel_spmd
from concourse.masks import make_identity
from concourse.vector_clock import ScopedClock, VectorClock

F32 = mybir.dt.float32
F32R = mybir.dt.float32r
BF16 = mybir.dt.bfloat16
AF = mybir.ActivationFunctionType
ALU = mybir.AluOpType

N_CORES = 8
B = 2048
D = 1024
L = 256
H = 16
HD = 64
S = 32
BT = 8            # batches per tile
TT = BT * S       # tokens per tile = 256
KB = D // 128     # 8 k-blocks
LB = L // 128     # 2 l-blocks
TB = TT // 128    # 2 token-blocks
NT = (B // N_CORES) * S // TT   # batch-tiles per core = 32


class SafeDrainTileContext(tile.TileContext):
    """TileContext that keeps every instruction at <=1 sem wait.

    This walrus build rejects >1 sem wait on an instruction ("Too many sync
    wait commands", e.g. Matmult S3_LW allows one). Excess waits are hoisted
    onto single-wait NOPs inserted just before the instruction on the same
    engine (engines execute their stream in order, so this is equivalent).
    The kernel-tail drain (emitted after lowering) gets the same treatment in
    _drain_and_barrier.
    """

    def _lower_ordered_insts(self, postordered_blocks):
        for bb, insts in postordered_blocks.items():
            new = []
            for inst in insts:
                si = inst.sync_info
                if si is not None and len(si.on_wait) > 1:
                    waits = list(si.on_wait)
                    for w in waits[:-1]:
                        nop = mybir.InstNoOp(
                            name=self.nc.get_next_instruction_name(),
                            sync_info=mybir.SyncInfo(on_wait=[w], on_update=[]),
                            engine=inst.engine,
                            bass_nofuse=True,
                            ins=[], outs=[],
                        )
                        new.append(nop)
                    inst.sync_info = mybir.SyncInfo(
                        on_wait=[waits[-1]], on_update=list(si.on_update)
                    )
                new.append(inst)
            postordered_blocks[bb] = new
        return super()._lower_ordered_insts(postordered_blocks)

    def _drain_and_barrier(self, tick_clock, wait_clock):
        gc = tick_clock.global_clock
        procs = [(i, t) for i, t in enumerate(gc) if t > 0]
        for i, t in procs:
            vec = VectorClock([0] * len(gc))
            vec.require_at_least(i, t)
            nop = self.nc.sync.nop(nofuse=True, hint="drain_split")
            wait_clock.add_sem_waits(nop.ins, ScopedClock({None: vec}))
        self.nc.sync.drain()
        self.nc.all_engine_barrier()
        assert self.sems is not None
        popped = self.nc._tile_sem_poison_stack.pop()
        assert popped is self._sem_poison
        self.nc.clear_and_free_semaphores(list(self.sems.allocated().values()))
        self.nc.all_engine_barrier()


class EngineBalancer:
    """Assign PSUM->SBUF copies to DVE or ACT, tracking estimated busy-ns."""

    def __init__(self, nc):
        self.nc = nc
        self.t_dve = 0.0
        self.t_act = 0.0

    def copy(self, out, in_, elems, two_byte=False):
        dve_ns = elems * 1.04 * (0.5 if two_byte else 1.0)
        act_ns = elems * 0.83
        if self.t_dve + dve_ns < self.t_act + act_ns:
            self.t_dve += dve_ns
            self.nc.vector.tensor_copy(out=out, in_=in_)
        else:
            self.t_act += act_ns
            self.nc.scalar.copy(out=out, in_=in_)

    def dve(self, ns):
        self.t_dve += ns

    def act(self, ns):
        self.t_act += ns


def build_kernel(nt=NT, name_suffix=""):
    """Build the per-core Bass program for nt 256-token batch-tiles."""
    n = nt * TT
    nc = bass.Bass(trn_type="TRN2", name="mla" + name_suffix)

    x = nc.dram_tensor("x", [n, D], F32R, kind="ExternalInput")
    mem = nc.dram_tensor("mem", [n, L], F32R, kind="ExternalInput")  # pre-scaled 0.5
    wq = nc.dram_tensor("wq", [D, D], F32R, kind="ExternalInput")
    wf = nc.dram_tensor("wf", [D, L], F32R, kind="ExternalInput")
    wuk = nc.dram_tensor("wuk", [L, D], F32R, kind="ExternalInput")
    wuv = nc.dram_tensor("wuv", [L, D], F32R, kind="ExternalInput")
    wo = nc.dram_tensor("wo", [D, D], BF16, kind="ExternalInput")
    out = nc.dram_tensor("out", [n, D], F32, kind="ExternalOutput")
    lat = nc.dram_tensor("lat", [n, L], F32, kind="ExternalOutput")

    with ExitStack() as ctx:
        tc = ctx.enter_context(SafeDrainTileContext(nc))
        wp = ctx.enter_context(tc.tile_pool(name="wp", bufs=1))
        io = ctx.enter_context(tc.tile_pool(name="io", bufs=2))
        ac = ctx.enter_context(tc.tile_pool(name="ac", bufs=2))
        ps_tr = ctx.enter_context(tc.tile_pool(name="ps_tr", bufs=2, space="PSUM"))
        ps_mm = ctx.enter_context(tc.tile_pool(name="ps_mm", bufs=4, space="PSUM"))
        ps_s = ctx.enter_context(tc.tile_pool(name="ps_s", bufs=2, space="PSUM"))

        # --- weights, resident in SBUF ---
        wq_sb = wp.tile([128, KB, D], F32R)
        nc.sync.dma_start(out=wq_sb, in_=wq.rearrange("(kb p) m -> p kb m", p=128))
        wf_sb = wp.tile([128, KB, L], F32R)
        nc.sync.dma_start(out=wf_sb, in_=wf.rearrange("(kb p) m -> p kb m", p=128))
        wuk_sb = wp.tile([128, LB, D], F32R)
        nc.sync.dma_start(out=wuk_sb, in_=wuk.rearrange("(kb p) m -> p kb m", p=128))
        wuv_sb = wp.tile([128, LB, D], F32R)
        nc.sync.dma_start(out=wuv_sb, in_=wuv.rearrange("(kb p) m -> p kb m", p=128))
        wo_sb = wp.tile([128, KB, D], BF16)
        nc.sync.dma_start(out=wo_sb, in_=wo.rearrange("(kb p) m -> p kb m", p=128))
        ident_f = wp.tile([128, 128], F32)
        make_identity(nc, ident_f)
        ident_r = wp.tile([128, 128], F32R)
        nc.vector.tensor_copy(out=ident_r, in_=ident_f)
        ident_bf = wp.tile([128, 128], BF16)
        make_identity(nc, ident_bf)

        for it in range(nt):
            t0 = it * TT
            eb = EngineBalancer(nc)

            # --- load x tile (token-major) ---
            x_tok = io.tile([128, TB, D], F32R, tag="x_tok")
            nc.sync.dma_start(
                out=x_tok, in_=x[t0:t0 + TT, :].rearrange("(a p) d -> p a d", p=128)
            )
            mem_tok = io.tile([128, TB, L], F32R, tag="mem_tok")
            nc.sync.dma_start(
                out=mem_tok, in_=mem[t0:t0 + TT, :].rearrange("(a p) d -> p a d", p=128)
            )

            # --- xT: [d_part, kb, token] via PE transposes ---
            xT = ac.tile([128, KB, TT], F32R, tag="xT")
            for db in range(KB):
                ptr = ps_tr.tile([128, TT], F32R, tag="tr")
                for a in range(TB):
                    nc.tensor.transpose(
                        ptr[:, 128 * a:128 * (a + 1)],
                        x_tok[:, a, 128 * db:128 * (db + 1)],
                        ident_r,
                    )
                eb.copy(xT[:, db, :], ptr, TT)

            # --- qT = Wq^T @ xT  (feature-major, bf16 out) ---
            qT = ac.tile([128, KB, TT], BF16, tag="qT")
            for mb in range(KB):
                pq = ps_mm.tile([128, 512], F32, tag="mm")
                for kb in range(KB):
                    nc.tensor.matmul(
                        pq[:, :TT],
                        wq_sb[:, kb, 128 * mb:128 * (mb + 1)],
                        xT[:, kb, :],
                        start=(kb == 0), stop=(kb == KB - 1),
                    )
                eb.copy(qT[:, mb, :], pq[:, :TT], TT)

            # --- latent: latT = Wf^T @ xT + memT ; lat_tok for output ---
            plat = ps_mm.tile([128, 512], F32, tag="mm")
            for lb in range(LB):
                for kb in range(KB):
                    nc.tensor.matmul(
                        plat[:, 256 * lb:256 * lb + TT],
                        wf_sb[:, kb, 128 * lb:128 * (lb + 1)],
                        xT[:, kb, :],
                        start=(kb == 0), stop=(kb == KB - 1),
                    )
            memT = ac.tile([128, LB, TT], F32R, tag="memT")
            for lb in range(LB):
                pmt = ps_tr.tile([128, TT], F32R, tag="tr")
                for a in range(TB):
                    nc.tensor.transpose(
                        pmt[:, 128 * a:128 * (a + 1)],
                        mem_tok[:, a, 128 * lb:128 * (lb + 1)],
                        ident_r,
                    )
                eb.copy(memT[:, lb, :], pmt, TT)
            latT = ac.tile([128, LB, TT], F32R, tag="latT")
            for lb in range(LB):
                nc.vector.tensor_tensor(
                    out=latT[:, lb, :], in0=plat[:, 256 * lb:256 * lb + TT],
                    in1=memT[:, lb, :], op=ALU.add,
                )
                eb.dve(TT * 1.04)
            lat_tok = io.tile([128, TB, L], F32, tag="lat_tok")
            for a in range(TB):
                plt = ps_tr.tile([128, TT], F32R, tag="tr")
                for lb in range(LB):
                    nc.tensor.transpose(
                        plt[:, 128 * lb:128 * (lb + 1)],
                        latT[:, lb, 128 * a:128 * (a + 1)],
                        ident_r,
                    )
                eb.copy(lat_tok[:, a, :], plt[:, :L], L)
            nc.sync.dma_start(
                out=lat[t0:t0 + TT, :].rearrange("(a p) d -> p a d", p=128),
                in_=lat_tok,
            )

            # --- kT = Wuk^T @ latT (bf16) ---
            kT = ac.tile([128, KB, TT], BF16, tag="kT")
            for mb in range(KB):
                pk = ps_mm.tile([128, 512], F32, tag="mm")
                for kb in range(LB):
                    nc.tensor.matmul(
                        pk[:, :TT],
                        wuk_sb[:, kb, 128 * mb:128 * (mb + 1)],
                        latT[:, kb, :],
                        start=(kb == 0), stop=(kb == LB - 1),
                    )
                eb.copy(kT[:, mb, :], pk[:, :TT], TT)

            # --- v token-major = latT.T @ Wuv (bf16) ---
            v_bf = ac.tile([128, TB, D], BF16, tag="v_bf")
            for tb in range(TB):
                for nb in range(2):
                    pv = ps_mm.tile([128, 512], F32, tag="mm")
                    for kb in range(LB):
                        nc.tensor.matmul(
                            pv,
                            latT[:, kb, 128 * tb:128 * (tb + 1)],
                            wuv_sb[:, kb, 512 * nb:512 * (nb + 1)],
                            start=(kb == 0), stop=(kb == LB - 1),
                        )
                    eb.copy(v_bf[:, tb, 512 * nb:512 * (nb + 1)], pv, 512)

            # --- scores + softmax ---
            # pair (b, h) -> psum_s[b//4][32*(h%4):+32, 32*(4*(h//4)+(b%4)):+32]
            exp_s = ac.tile([128, 2, 512], BF16, tag="exp_s")
            rsum = ac.tile([128, 2, 16], F32, tag="rsum")
            recip = ac.tile([128, 2, 16], F32, tag="recip")
            attn_bf = ac.tile([128, 2, 512], BF16, tag="attn_bf")
            for s in range(2):
                pss = ps_s.tile([128, 512], F32, tag="ps_s")
                for b4 in range(4):
                    b = 4 * s + b4
                    for h in range(H):
                        f = 4 * (h // 4) + b4
                        nc.tensor.matmul(
                            pss[32 * (h % 4):32 * (h % 4) + 32, 32 * f:32 * f + 32],
                            qT[64 * (h % 2):64 * (h % 2) + 64, h // 2, 32 * b:32 * b + 32],
                            kT[64 * (h % 2):64 * (h % 2) + 64, h // 2, 32 * b:32 * b + 32],
                            start=True, stop=True,
                            tile_position=(64 * (h % 2), 32 * (h % 4)),
                        )
                nc.scalar.activation(exp_s[:, s, :], pss, AF.Exp, scale=1.0 / 32.0)
                eb.act(512 * 0.83 + 1283)
                nc.vector.tensor_reduce(
                    out=rsum[:, s, :],
                    in_=exp_s[:, s, :].rearrange("p (g z) -> p g z", g=16),
                    op=ALU.add, axis=mybir.AxisListType.X,
                )
                eb.dve(512 * 0.52)
            nc.vector.reciprocal(recip.rearrange("p a b -> p (a b)"),
                                 rsum.rearrange("p a b -> p (a b)"))
            eb.dve(64.0)
            for s in range(2):
                rb = bass.AP(
                    tensor=recip.tensor, offset=recip[:, s, :].offset,
                    ap=[recip.ap[0], [1, 16], [0, 32]],
                )
                nc.vector.tensor_tensor(
                    out=attn_bf[:, s, :].rearrange("p (g z) -> p g z", g=16),
                    in0=exp_s[:, s, :].rearrange("p (g z) -> p g z", g=16),
                    in1=rb, op=ALU.mult,
                )
                eb.dve(512 * 1.04)

            # --- attnT via PE transpose of [128,128] sub-tiles ---
            # attnT[32*(b%4)+sk, b//4, h//4, 32*(h%4)+sq]
            attnT = ac.tile([128, 2, 4, 128], BF16, tag="attnT")
            for s in range(2):
                for g in range(4):
                    pat = ps_tr.tile([128, 128], BF16, tag="tr")
                    nc.tensor.transpose(
                        pat, attn_bf[:, s, 128 * g:128 * (g + 1)], ident_bf,
                    )
                    eb.copy(attnT[:, s, g, :], pat, 128, two_byte=True)

            # --- ctx = attn @ vh, packed for the out-transpose ---
            # psum_c[j=h//4][32*(b%4), ((h//2)%2)*256+(b//4)*128+(h%2)*64]
            ctx_sb = ac.tile([128, KB, TB, 128], BF16, tag="ctx_sb")
            for j in range(4):
                pc = ps_s.tile([128, 512], F32, tag="ps_s")
                for hh in range(4):
                    h = 4 * j + hh
                    for b in range(BT):
                        off = ((h // 2) % 2) * 256 + (b // 4) * 128 + (h % 2) * 64
                        nc.tensor.matmul(
                            pc[32 * (b % 4):32 * (b % 4) + 32, off:off + 64],
                            attnT[32 * (b % 4):32 * (b % 4) + 32, b // 4, h // 4, 32 * (h % 4):32 * (h % 4) + 32],
                            v_bf[32 * (b % 4):32 * (b % 4) + 32, b // 4, 64 * h:64 * h + 64],
                            start=True, stop=True,
                            tile_position=(32 * (b % 4), 32 * (b % 4)),
                        )
                eb.copy(
                    ctx_sb[:, 2 * j:2 * j + 2, :, :].rearrange("p u b z -> p (u b z)"),
                    pc, 512,
                )

            # --- ctxT ---
            ctxT = ac.tile([128, KB, TT], BF16, tag="ctxT")
            for u in range(KB):
                pct = ps_tr.tile([128, 256], BF16, tag="tr")
                for b2 in range(TB):
                    nc.tensor.transpose(
                        pct[:, 128 * b2:128 * (b2 + 1)], ctx_sb[:, u, b2, :], ident_bf,
                    )
                eb.copy(ctxT[:, u, :], pct, 256, two_byte=True)

            # --- out = ctxT.T @ Wo (token-major) ---
            out_sb = io.tile([128, TB, D], F32, tag="out_sb")
            for tb in range(TB):
                for nb in range(2):
                    po = ps_mm.tile([128, 512], F32, tag="mm")
                    for kb in range(KB):
                        nc.tensor.matmul(
                            po,
                            ctxT[:, kb, 128 * tb:128 * (tb + 1)],
                            wo_sb[:, kb, 512 * nb:512 * (nb + 1)],
                            start=(kb == 0), stop=(kb == KB - 1),
                        )
                    eb.copy(out_sb[:, tb, 512 * nb:512 * (nb + 1)], po, 512)
            nc.sync.dma_start(
                out=out[t0:t0 + TT, :].rearrange("(a p) d -> p a d", p=128),
                in_=out_sb,
            )

    return nc


_NC_CACHE = {}


def _get_nc(nt=NT):
    if nt not in _NC_CACHE:
        _NC_CACHE[nt] = build_kernel(nt)
    return _NC_CACHE[nt]


def kernel(x, memory, W_q, W_k, W_v, W_o, W_down_kv, W_up_k, W_up_v):
    x = np.asarray(x, dtype=np.float32)
    memory = np.asarray(memory, dtype=np.float32)
    W_q = np.asarray(W_q, dtype=np.float32)
    W_o = np.asarray(W_o, dtype=np.float32)

    # host-side weight folding (float64 for accuracy)
    wf = (
        0.5 * (np.asarray(W_k, np.float64) + np.asarray(W_v, np.float64))
        @ np.asarray(W_down_kv, np.float64)
    ).astype(np.float32)
    wuk = np.asarray(W_up_k, dtype=np.float32)
    wuv = np.asarray(W_up_v, dtype=np.float32)
    wo_bf = W_o.astype(ml_dtypes.bfloat16)
    mem_half = (0.5 * memory).reshape(B * S, L)
    x_flat = x.reshape(B * S, D)

    n_shard = (B // N_CORES) * S  # 8192 rows
    nc = _get_nc()
    in_maps = []
    for c in range(N_CORES):
        sl = slice(c * n_shard, (c + 1) * n_shard)
        in_maps.append({
            "x": np.ascontiguousarray(x_flat[sl]),
            "mem": np.ascontiguousarray(mem_half[sl]),
            "wq": W_q, "wf": wf, "wuk": wuk, "wuv": wuv, "wo": wo_bf,
        })
    res = run_bass_kernel_spmd(nc, in_maps, core_ids=list(range(N_CORES)))
    out = np.concatenate([res.results[c]["out"] for c in range(N_CORES)], axis=0)
    lat = np.concatenate([res.results[c]["lat"] for c in range(N_CORES)], axis=0)
    return out.reshape(B, S, D), lat.reshape(B, S, L)
